# revision 12
# baseline (speedup 1.0000x reference)
"""Trainium2 Bass kernel for a dense transformer block (B=2, T=2048, C=1024, 16 heads).

Sharding: core = 4*b + g  (b = batch, g = head-group / row-quarter).
  Attention: tensor-parallel over 4 heads per core within each batch group.
  One bf16 ReduceScatter turns attn-proj partial sums into per-core row shards.
  MLP: row-parallel (512 rows per core, full weights).

Attention-phase activations live transposed ([feature, token]) so every matmul
contracts over the partition dim without explicit transposes:
  - LN1 stats via ones-vector matmuls; mean-subtraction + LN bias folded into
    augmented weight rows; gamma folded into weights host-side.
  - softmax without max-subtraction (scores are provably bounded ~|2|);
    denominators come from an appended ones-column in the AV stationary.
  - causal masking via gpsimd affine_select on diagonal-crossing tiles only.
"""

import os
import numpy as np

B, T, C = 2, 2048, 1024
H, D, FF = 16, 64, 4096
N_CORES, G = 8, 4          # G cores per batch group
HPC = H // G               # heads per core
ROWS = T // G              # MLP rows per core
NCT = C // 128             # 8 c-tiles
CHUNK = 512                # query-chunk width
NCHUNK = T // CHUNK
EPS = 1e-5
QSCALE = float(1.0 / np.sqrt(D))

_cache = {}


def _patch_tile_drain():
    """This walrus build rejects >1 sem-wait on CTRL-class instructions; spread
    the TileContext tail-drain waits across single-wait SP nops."""
    import concourse.tile as tile
    from concourse import mybir
    from concourse.vector_clock import ScopedClock

    if getattr(tile.TileContext, "_drain_patched", False):
        return

    def _drain_and_barrier(self, tick_clock, wait_clock):
        nc = self.nc
        probe = nc.sync.nop()
        wait_clock.add_sem_waits(probe.ins, ScopedClock({None: tick_clock.global_clock}))
        waits = list(probe.ins.sync_info.on_wait) if probe.ins.sync_info else []
        probe.ins.sync_info = mybir.SyncInfo(on_wait=waits[:1], on_update=[])
        for w in waits[1:]:
            nop = nc.sync.nop()
            nop.ins.sync_info = mybir.SyncInfo(on_wait=[w], on_update=[])
        nc.sync.drain()
        nc.all_engine_barrier()
        assert self.sems is not None
        popped = nc._tile_sem_poison_stack.pop()
        assert popped is self._sem_poison
        nc.clear_and_free_semaphores(list(self.sems.allocated().values()))
        nc.all_engine_barrier()

    tile.TileContext._drain_and_barrier = _drain_and_barrier
    tile.TileContext._drain_patched = True


def _bcast_ap(bass, dram_ap, nparts):
    """DRAM AP replicated across nparts partitions (outer step 0)."""
    return bass.AP(tensor=dram_ap.tensor, offset=dram_ap.offset,
                   ap=[[0, nparts]] + [list(p) for p in dram_ap.ap])


def _split_excess_waits(nc, mybir, maxw=1):
    """walrus in this image rejects instructions carrying more than one sem
    wait; hoist the excess onto same-engine nops placed just before."""
    spare = []

    def make_nop(engine):
        if not spare:
            cur = nc.cur_bb.bb
            n0 = len(cur.instructions)
            for _ in range(64):
                nc.engines[engine].nop()
            insts = list(cur.instructions)
            spare.extend(insts[n0:])
            cur.instructions = insts[:n0]
        n = spare.pop()
        n.engine = engine
        return n

    for f in nc.m.functions:
        for bb in f.blocks:
            insts = list(bb.instructions)
            out = []
            changed = False
            for ins in insts:
                si = ins.sync_info
                if si and si.on_wait and len(si.on_wait) > maxw:
                    waits = list(si.on_wait)
                    for w in waits[maxw:]:
                        nop = make_nop(ins.engine)
                        nop.sync_info = mybir.SyncInfo(on_wait=[w], on_update=[])
                        out.append(nop)
                    ins.sync_info = mybir.SyncInfo(
                        on_wait=waits[:maxw],
                        on_update=list(si.on_update or []))
                    changed = True
                out.append(ins)
            if changed:
                bb.instructions = out


def _build():
    import concourse.bass as bass
    import concourse.tile as tile
    from concourse import mybir
    from concourse.masks import make_identity

    _patch_tile_drain()
    dt = mybir.dt
    AF = mybir.ActivationFunctionType
    ALU = mybir.AluOpType

    nc = bass.Bass("TRN2", target_bir_lowering=False, debug=False,
                   num_devices=N_CORES)

    # ---- per-core DRAM parameters (host supplies each core's shard) ----
    xT_d = nc.dram_tensor("xT", [C, T], dt.bfloat16, kind="ExternalInput")
    wqkv_d = nc.dram_tensor("wqkv", [C + 2, 3 * 64 * HPC], dt.bfloat16, kind="ExternalInput")
    wproj_d = nc.dram_tensor("wproj", [64 * HPC, C], dt.bfloat16, kind="ExternalInput")
    xrows_d = nc.dram_tensor("xrows", [ROWS, C], dt.float32, kind="ExternalInput")
    wfc_d = nc.dram_tensor("wfc", [C, FF], dt.bfloat16, kind="ExternalInput")
    bfc_d = nc.dram_tensor("bfc", [128, FF // 128], dt.float32, kind="ExternalInput")
    wmlp_d = nc.dram_tensor("wmlp", [FF + 128, C], dt.bfloat16, kind="ExternalInput")
    out_d = nc.dram_tensor("out", [ROWS, C], dt.float32, kind="ExternalOutput")

    # internal DRAM
    cc_in = nc.dram_tensor("cc_in", [T, C], dt.bfloat16)
    cc_out = nc.dram_tensor("cc_out", [ROWS, C], dt.bfloat16)
    murs_dr = nc.dram_tensor("murs_dr", [T], dt.bfloat16)
    rsig_dr = nc.dram_tensor("rsig_dr", [T], dt.bfloat16)

    NFT = FF // 128            # 32 f-tiles
    NFT_AUG = NFT + 1          # + bias row-tile

    with tile.TileContext(nc) as tc:
        with (
            tc.tile_pool(name="persist", bufs=1) as persist,
            tc.tile_pool(name="bouncep", bufs=3) as bouncep,
            tc.tile_pool(name="dramp", bufs=3, space="DRAM") as dramp,
            tc.tile_pool(name="qkvp", bufs=1) as qkvp,
        ):
            # ---------- persistent SBUF ----------
            wqkv_sb = persist.tile([128, NCT + 1, 3 * 64 * HPC], dt.bfloat16)
            wproj_sb = persist.tile([128, 2, C], dt.bfloat16)
            ident = persist.tile([128, 128], dt.bfloat16)
            ones_col = persist.tile([128, 1], dt.bfloat16)
            zero_col = persist.tile([128, 1], dt.float32)
            eps_col = persist.tile([128, 1], dt.float32)
            bfc_sb = persist.tile([128, FF // 128], dt.float32)
            q_sb = qkvp.tile([128, 2, T], dt.bfloat16)
            k_sb = qkvp.tile([128, 2, T], dt.bfloat16)
            v_sb = qkvp.tile([128, T // 128, (D + 1) * HPC], dt.bfloat16)

            make_identity(nc, ident[:])
            nc.vector.memset(zero_col[:], 0.0)
            nc.vector.memset(eps_col[:], EPS)
            nc.vector.memset(ones_col[:], 1.0)
            nc.sync.dma_start(wproj_sb[:, 0, :], wproj_d[0:128, :])
            nc.sync.dma_start(wproj_sb[:, 1, :], wproj_d[128:256, :])
            nc.sync.dma_start(bfc_sb[:], bfc_d[:])
            for ci in range(NCT):
                nc.sync.dma_start(wqkv_sb[:, ci, :], wqkv_d[128 * ci:128 * (ci + 1), :])
            nc.sync.dma_start(wqkv_sb[0:2, NCT, :], wqkv_d[C:C + 2, :])
            # ones column of v (col D of each head slot)
            vview = v_sb[:].rearrange("p t (h e) -> p t h e", h=HPC)
            nc.vector.memset(vview[:, :, :, D:D + 1], 1.0)

            with tc.tile_pool(name="xtp", bufs=1) as xtp, \
                 tc.tile_pool(name="statp", bufs=1) as statp:
                xt_sb = xtp.tile([128, NCT, T], dt.bfloat16)
                for ci in range(NCT):
                    nc.sync.dma_start(xt_sb[:, ci, :], xT_d[128 * ci:128 * (ci + 1), :])
                aux = statp.tile([2, T], dt.bfloat16)
                rsig_bc = statp.tile([128, T], dt.bfloat16)
                stats_f = statp.tile([1, 3, T], dt.float32)   # mu / work / work2
                stats_b = statp.tile([1, 2, T], dt.bfloat16)  # murs, rsig
                nc.vector.memset(aux[:], 1.0)

                # ---------- LN1 stats: column sums / sumsq via ones-matmul ----------
                with (
                    tc.tile_pool(name="sqp", bufs=2) as sqp,
                    tc.tile_pool(name="psS", bufs=1, space="PSUM") as psS,
                ):
                    ps_s = [psS.tile([1, CHUNK], dt.float32, tag=f"s{c}",
                                     name=f"ps_s{c}") for c in range(NCHUNK)]
                    ps_q = [psS.tile([1, CHUNK], dt.float32, tag=f"q{c}",
                                     name=f"ps_q{c}") for c in range(NCHUNK)]
                    for ci in range(NCT):
                        for half in range(2):
                            hsl = slice(1024 * half, 1024 * (half + 1))
                            sq = sqp.tile([128, 1024], dt.bfloat16)
                            nc.vector.tensor_mul(sq[:], xt_sb[:, ci, hsl], xt_sb[:, ci, hsl])
                            for c2 in range(2):
                                c = 2 * half + c2
                                sl = slice(CHUNK * c, CHUNK * (c + 1))
                                nc.tensor.matmul(ps_s[c][:], ones_col[:], xt_sb[:, ci, sl],
                                                 start=(ci == 0), stop=(ci == NCT - 1))
                                nc.tensor.matmul(ps_q[c][:], ones_col[:],
                                                 sq[:, CHUNK * c2:CHUNK * (c2 + 1)],
                                                 start=(ci == 0), stop=(ci == NCT - 1))
                    for c in range(NCHUNK):
                        sl = slice(CHUNK * c, CHUNK * (c + 1))
                        nc.vector.tensor_scalar_mul(stats_f[:, 0, sl], ps_s[c][:], 1.0 / C)
                        nc.vector.tensor_scalar_mul(stats_f[:, 1, sl], ps_q[c][:], 1.0 / C)
                mu1 = stats_f[:, 0, :]
                nc.vector.tensor_mul(stats_f[:, 2, :], mu1, mu1)              # musq
                nc.vector.tensor_sub(stats_f[:, 1, :], stats_f[:, 1, :], stats_f[:, 2, :])
                nc.scalar.activation(stats_f[:, 1, :], stats_f[:, 1, :], AF.Sqrt,
                                     bias=eps_col[0:1, :])
                nc.vector.reciprocal(stats_f[:, 2, :], stats_f[:, 1, :])      # rsig
                nc.vector.tensor_mul(stats_b[:, 0, :], mu1, stats_f[:, 2, :])  # murs
                nc.vector.tensor_copy(stats_b[:, 1, :], stats_f[:, 2, :])
                nc.sync.dma_start(murs_dr[:], stats_b[0:1, 0, :])
                nc.sync.dma_start(rsig_dr[:], stats_b[0:1, 1, :])
                nc.sync.dma_start(aux[0:1, :], murs_dr[:])
                nc.sync.dma_start(rsig_bc[:], _bcast_ap(bass, rsig_dr[:], 128))
                # scale columns: xs^T = x^T * rsig
                for ci in range(NCT):
                    nc.vector.tensor_mul(xt_sb[:, ci, :], xt_sb[:, ci, :], rsig_bc[:])

                # ---------- QKV ----------
                def contraction(ci):
                    if ci < NCT:
                        return xt_sb[:, ci, :], wqkv_sb[:, ci, :]
                    return aux[:], wqkv_sb[0:2, NCT, :]

                with tc.tile_pool(name="psQ", bufs=2, space="PSUM") as psQ:
                    for blk, dst in ((0, q_sb), (1, k_sb)):
                        for s in range(2):
                            cols = slice(256 * blk + 128 * s, 256 * blk + 128 * (s + 1))
                            for c in range(NCHUNK):
                                sl = slice(CHUNK * c, CHUNK * (c + 1))
                                ps = psQ.tile([128, CHUNK], dt.float32, tag="qk")
                                for ci in range(NCT + 1):
                                    act, w = contraction(ci)
                                    nc.tensor.matmul(ps[:], w[:, cols], act[:, sl],
                                                     start=(ci == 0), stop=(ci == NCT))
                                nc.scalar.copy(dst[:, s, sl], ps[:])
                    for tt in range(T // 128):          # v, natural layout
                        tsl = slice(128 * tt, 128 * (tt + 1))
                        ps = psQ.tile([128, 256], dt.float32, tag="v")
                        for ci in range(NCT + 1):
                            act, w = contraction(ci)
                            nc.tensor.matmul(ps[:], act[:, tsl], w[:, 512:768],
                                             start=(ci == 0), stop=(ci == NCT))
                        nc.scalar.copy(vview[:, tt, :, 0:D],
                                       ps[:].rearrange("p (h e) -> p h e", e=D))

            # xt/stats freed; prefetch MLP weights during attention
            with tc.tile_pool(name="mlpw", bufs=1) as mlpw, \
                 tc.tile_pool(name="wfcp", bufs=2) as wfcp:
                wmlp_sb = mlpw.tile([128, NFT_AUG, C], dt.bfloat16)
                for ft in range(NFT_AUG):
                    nc.sync.dma_start(wmlp_sb[:, ft, :], wmlp_d[128 * ft:128 * (ft + 1), :])

                # ---------- attention ----------
                with (
                    tc.tile_pool(name="attnp", bufs=1) as attnp,
                    tc.tile_pool(name="awork", bufs=2) as awork,
                    tc.tile_pool(name="expp", bufs=3) as expp,
                    tc.tile_pool(name="psA", bufs=2, space="PSUM") as psA,
                    tc.tile_pool(name="psY", bufs=1, space="PSUM") as psY,
                    tc.tile_pool(name="psP", bufs=2, space="PSUM") as psP,
                ):
                    y_sb = attnp.tile([128, 2, T], dt.bfloat16)
                    for c in range(NCHUNK):
                        i0 = CHUNK * c
                        isl = slice(i0, i0 + CHUNK)
                        njt = i0 // 128 + CHUNK // 128
                        for hp in range(2):              # head pairs share PE rows
                            ys = [psY.tile([D + 1, CHUNK], dt.float32, tag=f"y{u}",
                                           name=f"ys{u}") for u in range(2)]
                            for jt in range(njt):
                                jsl = slice(128 * jt, 128 * (jt + 1))
                                ess = []
                                for u in range(2):
                                    r = slice(64 * u, 64 * (u + 1))
                                    sp = psA.tile([128, CHUNK], dt.float32, tag=f"s{u}")
                                    nc.tensor.matmul(sp[:], k_sb[r, hp, jsl],
                                                     q_sb[r, hp, isl],
                                                     start=True, stop=True)
                                    es = expp.tile([128, CHUNK], dt.bfloat16, tag=f"e{u}")
                                    nc.scalar.activation(es[:], sp[:], AF.Exp,
                                                         bias=zero_col[:])
                                    if 128 * jt >= i0:   # diagonal-crossing tile
                                        nc.gpsimd.affine_select(
                                            out=es[:], in_=es[:], compare_op=ALU.is_ge,
                                            fill=0.0, base=i0 - 128 * jt,
                                            channel_multiplier=-1, pattern=[[1, CHUNK]])
                                    ess.append(es)
                                for u in range(2):
                                    h = 2 * hp + u
                                    nc.tensor.matmul(
                                        ys[u][:],
                                        v_sb[:, jt, (D + 1) * h:(D + 1) * (h + 1)],
                                        ess[u][:],
                                        start=(jt == 0), stop=(jt == njt - 1))
                            for u in range(2):
                                rc = awork.tile([D + 1, CHUNK], dt.float32, tag="rc")
                                nc.vector.reciprocal(rc[D:D + 1, :], ys[u][D:D + 1, :])
                                rcbf = awork.tile([D + 1, CHUNK], dt.bfloat16, tag="rcbf")
                                nc.vector.tensor_copy(rcbf[D:D + 1, :], rc[D:D + 1, :])
                                drc = dramp.tile([CHUNK], dt.bfloat16, tag="drc")
                                nc.sync.dma_start(drc[:], rcbf[D:D + 1, :])
                                rcb = awork.tile([D, CHUNK], dt.bfloat16, tag="rcb")
                                nc.sync.dma_start(rcb[:], _bcast_ap(bass, drc[:], D))
                                if u == 0:
                                    nc.vector.tensor_mul(y_sb[0:D, hp, isl],
                                                         ys[u][0:D, :], rcb[:])
                                else:
                                    yn = awork.tile([D, CHUNK], dt.bfloat16, tag="yn")
                                    nc.vector.tensor_mul(yn[:], ys[u][0:D, :], rcb[:])
                                    nc.sync.dma_start(y_sb[D:2 * D, hp, isl], yn[:])
                        # proj (natural out) for this chunk's rows
                        for tt4 in range(CHUNK // 128):
                            t0 = i0 + 128 * tt4
                            for cc in range(2):
                                csl = slice(512 * cc, 512 * (cc + 1))
                                pp = psP.tile([128, 512], dt.float32, tag="pj")
                                for s in range(2):
                                    nc.tensor.matmul(pp[:], y_sb[:, s, t0:t0 + 128],
                                                     wproj_sb[:, s, csl],
                                                     start=(s == 0), stop=(s == 1))
                                ob = bouncep.tile([128, 512], dt.bfloat16, tag="ob")
                                nc.scalar.copy(ob[:], pp[:])
                                nc.sync.dma_start(cc_in[t0:t0 + 128, csl], ob[:])

                # ---------- ReduceScatter over the 4-core batch group ----------
                nc.gpsimd.collective_compute(
                    "ReduceScatter", mybir.AluOpType.add,
                    replica_groups=[[0, 1, 2, 3], [4, 5, 6, 7]],
                    ins=[cc_in.ap().opt()],
                    outs=[cc_out.ap().opt()],
                )

                # ---------- MLP on own ROWS ----------
                with tc.tile_pool(name="mwork", bufs=2) as mwork, \
                     tc.tile_pool(name="mlp2", bufs=1) as mlp2, \
                     tc.tile_pool(name="psM", bufs=2, space="PSUM") as psM:
                    x2nT_sb = mlp2.tile([128, NCT, ROWS], dt.bfloat16)
                    h_sb = mlp2.tile([128, NFT_AUG, ROWS], dt.bfloat16)
                    for rt in range(ROWS // 128):
                        rsl = slice(128 * rt, 128 * (rt + 1))
                        xr = mwork.tile([128, C], dt.float32, tag="xr")
                        rs = mwork.tile([128, C], dt.bfloat16, tag="rs")
                        nc.sync.dma_start(xr[:], xrows_d[rsl, :])
                        nc.sync.dma_start(rs[:], cc_out[rsl, :])
                        x2t = mwork.tile([128, C], dt.float32, tag="x2t")
                        nc.vector.tensor_add(x2t[:], xr[:], rs[:])
                        # LN2 (natural): bn_stats over free dim
                        st = mwork.tile([128, 2, 6], dt.float32, tag="st")
                        mv = mwork.tile([128, 2], dt.float32, tag="mv")
                        sd = mwork.tile([128, 2], dt.float32, tag="sd")
                        x2v = x2t[:].rearrange("p (s n) -> p s n", s=2)
                        for s in range(2):
                            nc.vector.bn_stats(st[:, s, :], x2v[:, s, :])
                        nc.vector.bn_aggr(mv[:], st[:])
                        nc.scalar.activation(sd[:, 0:1], mv[:, 1:2], AF.Sqrt,
                                             bias=eps_col[:])
                        nc.vector.reciprocal(sd[:, 1:2], sd[:, 0:1])
                        x2n = mwork.tile([128, C], dt.bfloat16, tag="x2n")
                        nc.vector.tensor_scalar(x2n[:], x2t[:],
                                                mv[:, 0:1], sd[:, 1:2],
                                                op0=ALU.subtract, op1=ALU.mult)
                        for cb in range(NCT):
                            tp = psM.tile([128, 128], dt.bfloat16, tag="tp")
                            nc.tensor.transpose(tp[:], x2n[:, 128 * cb:128 * (cb + 1)],
                                                ident[:])
                            nc.scalar.copy(x2nT_sb[:, cb, rsl], tp[:])

                    # fc + gelu
                    for fchunk in range(FF // 512):
                        wf = wfcp.tile([128, NCT, 512], dt.bfloat16, tag="wf")
                        fsl = slice(512 * fchunk, 512 * (fchunk + 1))
                        for ci in range(NCT):
                            nc.sync.dma_start(wf[:, ci, :],
                                              wfc_d[128 * ci:128 * (ci + 1), fsl])
                        for ft4 in range(4):
                            ft = 4 * fchunk + ft4
                            hp_ps = psM.tile([128, ROWS], dt.float32, tag="hp")
                            for ci in range(NCT):
                                nc.tensor.matmul(hp_ps[:],
                                                 wf[:, ci, 128 * ft4:128 * (ft4 + 1)],
                                                 x2nT_sb[:, ci, :],
                                                 start=(ci == 0), stop=(ci == NCT - 1))
                            nc.scalar.activation(h_sb[:, ft, :], hp_ps[:], AF.Gelu,
                                                 bias=bfc_sb[:, ft:ft + 1])
                    # bias row-tile for mlp-proj: row0 = ones, rest 0
                    nc.vector.memset(h_sb[:, NFT, :], 0.0)
                    nc.vector.memset(h_sb[0:1, NFT, :], 1.0)

                    # mlp proj + residual
                    for rt in range(ROWS // 128):
                        rsl = slice(128 * rt, 128 * (rt + 1))
                        for cc in range(2):
                            csl = slice(512 * cc, 512 * (cc + 1))
                            mp = psM.tile([128, 512], dt.float32, tag="mp")
                            for ft in range(NFT_AUG):
                                nc.tensor.matmul(mp[:], h_sb[:, ft, rsl],
                                                 wmlp_sb[:, ft, csl],
                                                 start=(ft == 0), stop=(ft == NFT_AUG - 1))
                            xr2 = mwork.tile([128, 512], dt.float32, tag="xr2")
                            rs2 = mwork.tile([128, 512], dt.bfloat16, tag="rs2")
                            nc.sync.dma_start(xr2[:], xrows_d[rsl, csl])
                            nc.sync.dma_start(rs2[:], cc_out[rsl, csl])
                            fin = bouncep.tile([128, 512], dt.float32, tag="fin")
                            nc.vector.tensor_add(fin[:], mp[:], xr2[:])
                            nc.vector.tensor_add(fin[:], fin[:], rs2[:])
                            nc.sync.dma_start(out_d[rsl, csl], fin[:])

    _split_excess_waits(nc, mybir)
    return nc


def _get_nc():
    if "nc" not in _cache:
        _cache["nc"] = _build()
    return _cache["nc"]


def make_in_maps(inputs):
    import ml_dtypes
    bf16 = ml_dtypes.bfloat16
    x = np.asarray(inputs["x"], np.float32)
    w_qkv = np.asarray(inputs["w_qkv"], np.float32)
    w_attn_proj = np.asarray(inputs["w_attn_proj"], np.float32)
    ln1_w = np.asarray(inputs["ln1_w"], np.float32)
    ln1_b = np.asarray(inputs["ln1_b"], np.float32)
    ln2_w = np.asarray(inputs["ln2_w"], np.float32)
    ln2_b = np.asarray(inputs["ln2_b"], np.float32)
    w_fc = np.asarray(inputs["w_fc"], np.float32)
    b_fc = np.asarray(inputs["b_fc"], np.float32)
    w_mlp_proj = np.asarray(inputs["w_mlp_proj"], np.float32)
    b_mlp_proj = np.asarray(inputs["b_mlp_proj"], np.float32)

    wfc_in = (ln2_w[:, None] * w_fc).astype(bf16)
    bfc_aug = b_fc + ln2_b @ w_fc
    bfc_in = np.ascontiguousarray(bfc_aug.reshape(FF // 128, 128).T).astype(np.float32)
    wmlp_in = np.vstack([w_mlp_proj, b_mlp_proj[None, :],
                         np.zeros((127, C), np.float32)]).astype(bf16)

    in_maps = []
    for core in range(N_CORES):
        b, g = divmod(core, G)
        hsl = slice(256 * g, 256 * (g + 1))
        raw768 = np.concatenate([w_qkv[:, :C][:, hsl] * QSCALE,
                                 w_qkv[:, C:2 * C][:, hsl],
                                 w_qkv[:, 2 * C:][:, hsl]], axis=1)
        W768 = ln1_w[:, None] * raw768
        wqkv_in = np.vstack([W768, -W768.sum(0, keepdims=True),
                             (ln1_b @ raw768)[None, :]]).astype(bf16)
        rows = slice(ROWS * g, ROWS * (g + 1))
        in_maps.append({
            "xT": np.ascontiguousarray(x[b].T).astype(bf16),
            "wqkv": wqkv_in,
            "wproj": w_attn_proj[hsl, :].astype(bf16),
            "xrows": np.ascontiguousarray(x[b, rows, :]),
            "wfc": wfc_in,
            "bfc": bfc_in,
            "wmlp": wmlp_in,
        })
    return in_maps


def assemble_out(results):
    out = np.empty((B, T, C), np.float32)
    for core in range(N_CORES):
        b, g = divmod(core, G)
        out[b, ROWS * g:ROWS * (g + 1), :] = results[core]["out"]
    return out


def kernel(**inputs):
    from concourse.bass_utils import run_bass_kernel_spmd

    in_maps = make_in_maps(inputs)
    nc = _get_nc()
    trace = bool(os.environ.get("KERNEL_TRACE"))
    res = run_bass_kernel_spmd(nc, in_maps, core_ids=list(range(N_CORES)),
                               trace=trace)
    if trace:
        _cache["exec_time_ns"] = res.exec_time_ns
    return assemble_out(res.results)


if __name__ == "__main__":
    nc = _get_nc()
    print("built OK; instructions:", len(nc.inst_map))


# revision 14
# speedup vs baseline: 1.1914x; 1.1914x over previous
"""Trainium2 Bass kernel for a dense transformer block (B=2, T=2048, C=1024, 16 heads).

Sharding: core = 4*b + g  (b = batch, g = head-group / row-quarter).
  Attention: tensor-parallel over 4 heads per core within each batch group.
  One bf16 ReduceScatter turns attn-proj partial sums into per-core row shards.
  MLP: row-parallel (512 rows per core, full weights).

Attention-phase activations live transposed ([feature, token]) so every matmul
contracts over the partition dim without explicit transposes:
  - LN1 stats via ones-vector matmuls; mean-subtraction + LN bias folded into
    augmented weight rows; gamma folded into weights host-side.
  - softmax without max-subtraction (scores are provably bounded ~|2|);
    denominators come from an appended ones-column in the AV stationary.
  - causal masking via gpsimd affine_select on diagonal-crossing tiles only.
"""

import os
import numpy as np

B, T, C = 2, 2048, 1024
H, D, FF = 16, 64, 4096
N_CORES, G = 8, 4          # G cores per batch group
HPC = H // G               # heads per core
ROWS = T // G              # MLP rows per core
NCT = C // 128             # 8 c-tiles
CHUNK = 512                # query-chunk width
NCHUNK = T // CHUNK
EPS = 1e-5
QSCALE = float(1.0 / np.sqrt(D))

_cache = {}


def _patch_tile_drain():
    """This walrus build rejects >1 sem-wait on CTRL-class instructions; spread
    the TileContext tail-drain waits across single-wait SP nops."""
    import concourse.tile as tile
    from concourse import mybir
    from concourse.vector_clock import ScopedClock

    if getattr(tile.TileContext, "_drain_patched", False):
        return

    def _drain_and_barrier(self, tick_clock, wait_clock):
        nc = self.nc
        probe = nc.sync.nop()
        wait_clock.add_sem_waits(probe.ins, ScopedClock({None: tick_clock.global_clock}))
        waits = list(probe.ins.sync_info.on_wait) if probe.ins.sync_info else []
        probe.ins.sync_info = mybir.SyncInfo(on_wait=waits[:1], on_update=[])
        for w in waits[1:]:
            nop = nc.sync.nop()
            nop.ins.sync_info = mybir.SyncInfo(on_wait=[w], on_update=[])
        nc.sync.drain()
        nc.all_engine_barrier()
        assert self.sems is not None
        popped = nc._tile_sem_poison_stack.pop()
        assert popped is self._sem_poison
        nc.clear_and_free_semaphores(list(self.sems.allocated().values()))
        nc.all_engine_barrier()

    tile.TileContext._drain_and_barrier = _drain_and_barrier
    tile.TileContext._drain_patched = True


def _bcast_ap(bass, dram_ap, nparts):
    """DRAM AP replicated across nparts partitions (outer step 0)."""
    return bass.AP(tensor=dram_ap.tensor, offset=dram_ap.offset,
                   ap=[[0, nparts]] + [list(p) for p in dram_ap.ap])


def _split_excess_waits(nc, mybir, maxw=1):
    """walrus in this image rejects instructions carrying more than one sem
    wait; hoist the excess onto same-engine nops placed just before."""
    spare = []

    def make_nop(engine):
        if not spare:
            cur = nc.cur_bb.bb
            n0 = len(cur.instructions)
            for _ in range(64):
                nc.engines[engine].nop()
            insts = list(cur.instructions)
            spare.extend(insts[n0:])
            cur.instructions = insts[:n0]
        n = spare.pop()
        n.engine = engine
        return n

    for f in nc.m.functions:
        for bb in f.blocks:
            insts = list(bb.instructions)
            out = []
            changed = False
            for ins in insts:
                si = ins.sync_info
                if si and si.on_wait and len(si.on_wait) > maxw:
                    waits = list(si.on_wait)
                    for w in waits[maxw:]:
                        nop = make_nop(ins.engine)
                        nop.sync_info = mybir.SyncInfo(on_wait=[w], on_update=[])
                        out.append(nop)
                    ins.sync_info = mybir.SyncInfo(
                        on_wait=waits[:maxw],
                        on_update=list(si.on_update or []))
                    changed = True
                out.append(ins)
            if changed:
                bb.instructions = out


def _build():
    import concourse.bass as bass
    import concourse.tile as tile
    from concourse import mybir
    from concourse.masks import make_identity

    _patch_tile_drain()
    dt = mybir.dt
    AF = mybir.ActivationFunctionType
    ALU = mybir.AluOpType

    nc = bass.Bass("TRN2", target_bir_lowering=False, debug=False,
                   num_devices=N_CORES)

    # ---- per-core DRAM parameters (host supplies each core's shard) ----
    xT_d = nc.dram_tensor("xT", [C, T], dt.bfloat16, kind="ExternalInput")
    wqkv_d = nc.dram_tensor("wqkv", [C + 2, 3 * 64 * HPC], dt.bfloat16, kind="ExternalInput")
    wproj_d = nc.dram_tensor("wproj", [64 * HPC, C], dt.bfloat16, kind="ExternalInput")
    xrows_d = nc.dram_tensor("xrows", [ROWS, C], dt.float32, kind="ExternalInput")
    wfc_d = nc.dram_tensor("wfc", [C, FF], dt.bfloat16, kind="ExternalInput")
    bfc_d = nc.dram_tensor("bfc", [128, FF // 128], dt.float32, kind="ExternalInput")
    wmlp_d = nc.dram_tensor("wmlp", [FF + 128, C], dt.bfloat16, kind="ExternalInput")
    out_d = nc.dram_tensor("out", [ROWS, C], dt.float32, kind="ExternalOutput")

    # internal DRAM
    cc_in = [nc.dram_tensor(f"cc_in{c}", [CHUNK, C], dt.bfloat16)
             for c in range(NCHUNK)]
    cc_out = [nc.dram_tensor(f"cc_out{c}", [CHUNK // G, C], dt.bfloat16)
              for c in range(NCHUNK)]
    mu_dr = nc.dram_tensor("mu_dr", [T], dt.bfloat16)
    std_dr = nc.dram_tensor("std_dr", [T], dt.bfloat16)
    rsig_dr = nc.dram_tensor("rsig_dr", [T], dt.bfloat16)
    rsigf_dr = nc.dram_tensor("rsigf_dr", [T], dt.float32)

    NFT = FF // 128            # 32 f-tiles
    NFT_AUG = NFT + 1          # + bias row-tile

    with tile.TileContext(nc) as tc:
        with (
            tc.tile_pool(name="persist", bufs=1) as persist,
            tc.tile_pool(name="bouncep", bufs=3) as bouncep,
            tc.tile_pool(name="dramp", bufs=3, space="DRAM") as dramp,
            tc.tile_pool(name="qkvp", bufs=1) as qkvp,
        ):
            # ---------- persistent SBUF ----------
            wqkv_sb = persist.tile([128, NCT + 1, 3 * 64 * HPC], dt.bfloat16)
            wproj_sb = persist.tile([128, 2, C], dt.bfloat16)
            ident = persist.tile([128, 128], dt.bfloat16)
            ones_col = persist.tile([128, 1], dt.bfloat16)
            zero_col = persist.tile([128, 1], dt.float32)
            eps_col = persist.tile([128, 1], dt.float32)
            bfc_sb = persist.tile([128, FF // 128], dt.float32)
            q_sb = qkvp.tile([128, 2, T], dt.bfloat16)
            k_sb = qkvp.tile([128, 2, T], dt.bfloat16)
            v_sb = qkvp.tile([128, T // 128, (D + 1) * HPC], dt.bfloat16)

            make_identity(nc, ident[:])
            nc.vector.memset(zero_col[:], 0.0)
            nc.vector.memset(eps_col[:], EPS)
            nc.vector.memset(ones_col[:], 1.0)
            nc.sync.dma_start(wproj_sb[:, 0, :], wproj_d[0:128, :])
            nc.sync.dma_start(wproj_sb[:, 1, :], wproj_d[128:256, :])
            nc.sync.dma_start(bfc_sb[:], bfc_d[:])
            for ci in range(NCT):
                nc.sync.dma_start(wqkv_sb[:, ci, :], wqkv_d[128 * ci:128 * (ci + 1), :])
            nc.sync.dma_start(wqkv_sb[0:2, NCT, :], wqkv_d[C:C + 2, :])
            # ones column of v (col D of each head slot)
            vview = v_sb[:].rearrange("p t (h e) -> p t h e", h=HPC)
            nc.vector.memset(vview[:, :, :, D:D + 1], 1.0)

            with tc.tile_pool(name="xtp", bufs=1) as xtp, \
                 tc.tile_pool(name="statp", bufs=1) as statp:
                xt_sb = xtp.tile([128, NCT, T], dt.bfloat16)
                for ci in range(NCT):
                    nc.sync.dma_start(xt_sb[:, ci, :], xT_d[128 * ci:128 * (ci + 1), :])
                aux = statp.tile([2, T], dt.bfloat16)
                rsig_bc = statp.tile([128, T], dt.bfloat16)
                rsig_col = statp.tile([128, T // 128], dt.float32)
                stats_f = statp.tile([1, 3, T], dt.float32)   # mu / work / work2
                stats_b = statp.tile([1, 3, T], dt.bfloat16)  # mu, std, rsig

                # ---------- LN1 stats: column sums / sumsq via ones-matmul ----------
                with (
                    tc.tile_pool(name="sqp", bufs=2) as sqp,
                    tc.tile_pool(name="psS", bufs=1, space="PSUM") as psS,
                ):
                    ps_s = [psS.tile([1, CHUNK], dt.float32, tag=f"s{c}",
                                     name=f"ps_s{c}") for c in range(NCHUNK)]
                    ps_q = [psS.tile([1, CHUNK], dt.float32, tag=f"q{c}",
                                     name=f"ps_q{c}") for c in range(NCHUNK)]
                    for ci in range(NCT):
                        for half in range(2):
                            hsl = slice(1024 * half, 1024 * (half + 1))
                            sq = sqp.tile([128, 1024], dt.bfloat16)
                            nc.vector.tensor_mul(sq[:], xt_sb[:, ci, hsl], xt_sb[:, ci, hsl])
                            for c2 in range(2):
                                c = 2 * half + c2
                                sl = slice(CHUNK * c, CHUNK * (c + 1))
                                nc.tensor.matmul(ps_s[c][:], ones_col[:], xt_sb[:, ci, sl],
                                                 start=(ci == 0), stop=(ci == NCT - 1))
                                nc.tensor.matmul(ps_q[c][:], ones_col[:],
                                                 sq[:, CHUNK * c2:CHUNK * (c2 + 1)],
                                                 start=(ci == 0), stop=(ci == NCT - 1))
                    for c in range(NCHUNK):
                        sl = slice(CHUNK * c, CHUNK * (c + 1))
                        nc.vector.tensor_scalar_mul(stats_f[:, 0, sl], ps_s[c][:], 1.0 / C)
                        nc.vector.tensor_scalar_mul(stats_f[:, 1, sl], ps_q[c][:], 1.0 / C)
                mu1 = stats_f[:, 0, :]
                nc.vector.tensor_mul(stats_f[:, 2, :], mu1, mu1)              # musq
                nc.vector.tensor_sub(stats_f[:, 1, :], stats_f[:, 1, :], stats_f[:, 2, :])
                nc.scalar.activation(stats_f[:, 1, :], stats_f[:, 1, :], AF.Sqrt,
                                     bias=eps_col[0:1, :])                    # std
                nc.vector.reciprocal(stats_f[:, 2, :], stats_f[:, 1, :])      # rsig
                nc.vector.tensor_copy(stats_b[:, 0, :], mu1)
                nc.vector.tensor_copy(stats_b[:, 1, :], stats_f[:, 1, :])
                nc.vector.tensor_copy(stats_b[:, 2, :], stats_f[:, 2, :])
                nc.sync.dma_start(mu_dr[:], stats_b[0:1, 0, :])
                nc.sync.dma_start(std_dr[:], stats_b[0:1, 1, :])
                nc.sync.dma_start(rsig_dr[:], stats_b[0:1, 2, :])
                nc.sync.dma_start(rsigf_dr[:], stats_f[0:1, 2, :])
                nc.sync.dma_start(aux[0:1, :], mu_dr[:])
                nc.sync.dma_start(aux[1:2, :], std_dr[:])
                nc.sync.dma_start(rsig_bc[:], _bcast_ap(bass, rsig_dr[:], 128))
                rcol_src = bass.AP(tensor=rsigf_dr, offset=0,
                                   ap=[[1, 128], [128, T // 128]])
                nc.sync.dma_start(rsig_col[:], rcol_src)

                # ---------- QKV ----------
                def contraction(ci):
                    if ci < NCT:
                        return xt_sb[:, ci, :], wqkv_sb[:, ci, :]
                    return aux[:], wqkv_sb[0:2, NCT, :]

                with tc.tile_pool(name="psQ", bufs=2, space="PSUM") as psQ:
                    for blk, dst in ((0, q_sb), (1, k_sb)):
                        for s in range(2):
                            cols = slice(256 * blk + 128 * s, 256 * blk + 128 * (s + 1))
                            for c in range(NCHUNK):
                                sl = slice(CHUNK * c, CHUNK * (c + 1))
                                ps = psQ.tile([128, CHUNK], dt.float32, tag="qk")
                                for ci in range(NCT + 1):
                                    act, w = contraction(ci)
                                    nc.tensor.matmul(ps[:], w[:, cols], act[:, sl],
                                                     start=(ci == 0), stop=(ci == NCT))
                                nc.vector.tensor_mul(dst[:, s, sl], ps[:],
                                                     rsig_bc[:, sl])
                    for tt in range(T // 128):          # v, natural layout
                        tsl = slice(128 * tt, 128 * (tt + 1))
                        ps = psQ.tile([128, 256], dt.float32, tag="v")
                        for ci in range(NCT + 1):
                            act, w = contraction(ci)
                            nc.tensor.matmul(ps[:], act[:, tsl], w[:, 512:768],
                                             start=(ci == 0), stop=(ci == NCT))
                        nc.scalar.mul(vview[:, tt, :, 0:D],
                                      ps[:].rearrange("p (h e) -> p h e", e=D),
                                      rsig_col[:, tt:tt + 1])

            # xt/stats freed; prefetch MLP weights during attention
            with tc.tile_pool(name="mlpw", bufs=1) as mlpw, \
                 tc.tile_pool(name="wfcp", bufs=2) as wfcp:
                wmlp_sb = mlpw.tile([128, NFT_AUG, C], dt.bfloat16)
                for ft in range(NFT_AUG):
                    nc.sync.dma_start(wmlp_sb[:, ft, :], wmlp_d[128 * ft:128 * (ft + 1), :])

                # ---------- attention ----------
                with (
                    tc.tile_pool(name="attnp", bufs=1) as attnp,
                    tc.tile_pool(name="awork", bufs=2) as awork,
                    tc.tile_pool(name="expp", bufs=3) as expp,
                    tc.tile_pool(name="psA", bufs=2, space="PSUM") as psA,
                    tc.tile_pool(name="psY", bufs=2, space="PSUM") as psY,
                ):
                    y_sb = attnp.tile([128, 2, T], dt.bfloat16)
                    for c in range(NCHUNK):
                        i0 = CHUNK * c
                        isl = slice(i0, i0 + CHUNK)
                        njt = i0 // 128 + CHUNK // 128
                        for hp in range(2):              # head pairs share PE rows
                            ys = [psY.tile([D + 1, CHUNK], dt.float32, tag=f"y{u}",
                                           name=f"ys{u}") for u in range(2)]
                            for jt in range(njt):
                                jsl = slice(128 * jt, 128 * (jt + 1))
                                ess = []
                                for u in range(2):
                                    r = slice(64 * u, 64 * (u + 1))
                                    sp = psA.tile([128, CHUNK], dt.float32, tag=f"s{u}")
                                    nc.tensor.matmul(sp[:], k_sb[r, hp, jsl],
                                                     q_sb[r, hp, isl],
                                                     start=True, stop=True)
                                    es = expp.tile([128, CHUNK], dt.bfloat16, tag=f"e{u}")
                                    nc.scalar.activation(es[:], sp[:], AF.Exp,
                                                         bias=zero_col[:])
                                    if 128 * jt >= i0:   # diagonal-crossing tile
                                        nc.gpsimd.affine_select(
                                            out=es[:], in_=es[:], compare_op=ALU.is_ge,
                                            fill=0.0, base=i0 - 128 * jt,
                                            channel_multiplier=-1, pattern=[[1, CHUNK]])
                                    ess.append(es)
                                for u in range(2):
                                    h = 2 * hp + u
                                    nc.tensor.matmul(
                                        ys[u][:],
                                        v_sb[:, jt, (D + 1) * h:(D + 1) * (h + 1)],
                                        ess[u][:],
                                        start=(jt == 0), stop=(jt == njt - 1))
                            for u in range(2):
                                rc = awork.tile([D + 1, CHUNK], dt.float32, tag="rc")
                                nc.vector.reciprocal(rc[D:D + 1, :], ys[u][D:D + 1, :])
                                rcbf = awork.tile([D + 1, CHUNK], dt.bfloat16, tag="rcbf")
                                nc.vector.tensor_copy(rcbf[D:D + 1, :], rc[D:D + 1, :])
                                drc = dramp.tile([CHUNK], dt.bfloat16, tag="drc")
                                nc.sync.dma_start(drc[:], rcbf[D:D + 1, :])
                                rcb = awork.tile([D, CHUNK], dt.bfloat16, tag="rcb")
                                nc.sync.dma_start(rcb[:], _bcast_ap(bass, drc[:], D))
                                if u == 0:
                                    nc.vector.tensor_mul(y_sb[0:D, hp, isl],
                                                         ys[u][0:D, :], rcb[:])
                                else:
                                    yn = awork.tile([D, CHUNK], dt.bfloat16, tag="yn")
                                    nc.vector.tensor_mul(yn[:], ys[u][0:D, :], rcb[:])
                                    nc.sync.dma_start(y_sb[D:2 * D, hp, isl], yn[:])
                        # proj (natural out) for this chunk's rows
                        for tt4 in range(CHUNK // 128):
                            t0 = i0 + 128 * tt4
                            for cc in range(2):
                                csl = slice(512 * cc, 512 * (cc + 1))
                                pp = psA.tile([128, 512], dt.float32, tag="s0")
                                for s in range(2):
                                    nc.tensor.matmul(pp[:], y_sb[:, s, t0:t0 + 128],
                                                     wproj_sb[:, s, csl],
                                                     start=(s == 0), stop=(s == 1))
                                ob = bouncep.tile([128, 512], dt.bfloat16, tag="ob")
                                nc.scalar.copy(ob[:], pp[:])
                                nc.sync.dma_start(cc_in[c][128 * tt4:128 * (tt4 + 1), csl],
                                                  ob[:])
                        # ReduceScatter this chunk over the 4-core batch group
                        nc.gpsimd.collective_compute(
                            "ReduceScatter", mybir.AluOpType.add,
                            replica_groups=[[0, 1, 2, 3], [4, 5, 6, 7]],
                            ins=[cc_in[c].ap().opt()],
                            outs=[cc_out[c].ap().opt()],
                        )

                # ---------- MLP on own ROWS ----------
                with tc.tile_pool(name="mwork", bufs=2) as mwork, \
                     tc.tile_pool(name="mlp2", bufs=1) as mlp2, \
                     tc.tile_pool(name="psM", bufs=2, space="PSUM") as psM:
                    x2nT_sb = mlp2.tile([128, NCT, ROWS], dt.bfloat16)
                    h_sb = mlp2.tile([128, NFT_AUG, ROWS], dt.bfloat16)
                    for rt in range(ROWS // 128):
                        rsl = slice(128 * rt, 128 * (rt + 1))
                        xr = mwork.tile([128, C], dt.float32, tag="xr")
                        rs = mwork.tile([128, C], dt.bfloat16, tag="rs")
                        nc.sync.dma_start(xr[:], xrows_d[rsl, :])
                        nc.sync.dma_start(rs[:], cc_out[rt][:, :])
                        x2t = mwork.tile([128, C], dt.float32, tag="x2t")
                        nc.vector.tensor_add(x2t[:], xr[:], rs[:])
                        # LN2 (natural): bn_stats over free dim
                        st = mwork.tile([128, 2, 6], dt.float32, tag="st")
                        mv = mwork.tile([128, 2], dt.float32, tag="mv")
                        sd = mwork.tile([128, 2], dt.float32, tag="sd")
                        x2v = x2t[:].rearrange("p (s n) -> p s n", s=2)
                        for s in range(2):
                            nc.vector.bn_stats(st[:, s, :], x2v[:, s, :])
                        nc.vector.bn_aggr(mv[:], st[:])
                        nc.scalar.activation(sd[:, 0:1], mv[:, 1:2], AF.Sqrt,
                                             bias=eps_col[:])
                        nc.vector.reciprocal(sd[:, 1:2], sd[:, 0:1])
                        x2n = mwork.tile([128, C], dt.bfloat16, tag="x2n")
                        nc.vector.tensor_scalar(x2n[:], x2t[:],
                                                mv[:, 0:1], sd[:, 1:2],
                                                op0=ALU.subtract, op1=ALU.mult)
                        for cb in range(NCT):
                            tp = psM.tile([128, 128], dt.bfloat16, tag="tp")
                            nc.tensor.transpose(tp[:], x2n[:, 128 * cb:128 * (cb + 1)],
                                                ident[:])
                            nc.scalar.copy(x2nT_sb[:, cb, rsl], tp[:])

                    # fc + gelu
                    for fchunk in range(FF // 512):
                        wf = wfcp.tile([128, NCT, 512], dt.bfloat16, tag="wf")
                        fsl = slice(512 * fchunk, 512 * (fchunk + 1))
                        for ci in range(NCT):
                            nc.sync.dma_start(wf[:, ci, :],
                                              wfc_d[128 * ci:128 * (ci + 1), fsl])
                        for ft4 in range(4):
                            ft = 4 * fchunk + ft4
                            hp_ps = psM.tile([128, ROWS], dt.float32, tag="hp")
                            for ci in range(NCT):
                                nc.tensor.matmul(hp_ps[:],
                                                 wf[:, ci, 128 * ft4:128 * (ft4 + 1)],
                                                 x2nT_sb[:, ci, :],
                                                 start=(ci == 0), stop=(ci == NCT - 1))
                            nc.scalar.activation(h_sb[:, ft, :], hp_ps[:], AF.Gelu,
                                                 bias=bfc_sb[:, ft:ft + 1])
                    # bias row-tile for mlp-proj: row0 = ones, rest 0
                    nc.vector.memset(h_sb[:, NFT, :], 0.0)
                    nc.vector.memset(h_sb[0:1, NFT, :], 1.0)

                    # mlp proj + residual
                    for rt in range(ROWS // 128):
                        rsl = slice(128 * rt, 128 * (rt + 1))
                        for cc in range(2):
                            csl = slice(512 * cc, 512 * (cc + 1))
                            mp = psM.tile([128, 512], dt.float32, tag="mp")
                            for ft in range(NFT_AUG):
                                nc.tensor.matmul(mp[:], h_sb[:, ft, rsl],
                                                 wmlp_sb[:, ft, csl],
                                                 start=(ft == 0), stop=(ft == NFT_AUG - 1))
                            xr2 = mwork.tile([128, 512], dt.float32, tag="xr2")
                            rs2 = mwork.tile([128, 512], dt.bfloat16, tag="rs2")
                            nc.sync.dma_start(xr2[:], xrows_d[rsl, csl])
                            nc.sync.dma_start(rs2[:], cc_out[rt][:, csl])
                            fin = bouncep.tile([128, 512], dt.float32, tag="fin")
                            nc.vector.tensor_add(fin[:], mp[:], xr2[:])
                            nc.vector.tensor_add(fin[:], fin[:], rs2[:])
                            nc.sync.dma_start(out_d[rsl, csl], fin[:])

    _split_excess_waits(nc, mybir)
    return nc


def _get_nc():
    if "nc" not in _cache:
        _cache["nc"] = _build()
    return _cache["nc"]


def make_in_maps(inputs):
    import ml_dtypes
    bf16 = ml_dtypes.bfloat16
    x = np.asarray(inputs["x"], np.float32)
    w_qkv = np.asarray(inputs["w_qkv"], np.float32)
    w_attn_proj = np.asarray(inputs["w_attn_proj"], np.float32)
    ln1_w = np.asarray(inputs["ln1_w"], np.float32)
    ln1_b = np.asarray(inputs["ln1_b"], np.float32)
    ln2_w = np.asarray(inputs["ln2_w"], np.float32)
    ln2_b = np.asarray(inputs["ln2_b"], np.float32)
    w_fc = np.asarray(inputs["w_fc"], np.float32)
    b_fc = np.asarray(inputs["b_fc"], np.float32)
    w_mlp_proj = np.asarray(inputs["w_mlp_proj"], np.float32)
    b_mlp_proj = np.asarray(inputs["b_mlp_proj"], np.float32)

    wfc_in = (ln2_w[:, None] * w_fc).astype(bf16)
    bfc_aug = b_fc + ln2_b @ w_fc
    bfc_in = np.ascontiguousarray(bfc_aug.reshape(FF // 128, 128).T).astype(np.float32)
    wmlp_in = np.vstack([w_mlp_proj, b_mlp_proj[None, :],
                         np.zeros((127, C), np.float32)]).astype(bf16)

    in_maps = []
    for core in range(N_CORES):
        b, g = divmod(core, G)
        hsl = slice(256 * g, 256 * (g + 1))
        raw768 = np.concatenate([w_qkv[:, :C][:, hsl] * QSCALE,
                                 w_qkv[:, C:2 * C][:, hsl],
                                 w_qkv[:, 2 * C:][:, hsl]], axis=1)
        W768 = ln1_w[:, None] * raw768
        wqkv_in = np.vstack([W768, -W768.sum(0, keepdims=True),
                             (ln1_b @ raw768)[None, :]]).astype(bf16)
        rowidx = np.concatenate([np.arange(CHUNK * c + 128 * g,
                                           CHUNK * c + 128 * (g + 1))
                                 for c in range(NCHUNK)])
        in_maps.append({
            "xT": np.ascontiguousarray(x[b].T).astype(bf16),
            "wqkv": wqkv_in,
            "wproj": w_attn_proj[hsl, :].astype(bf16),
            "xrows": np.ascontiguousarray(x[b, rowidx, :]),
            "wfc": wfc_in,
            "bfc": bfc_in,
            "wmlp": wmlp_in,
        })
    return in_maps


def assemble_out(results):
    out = np.empty((B, T, C), np.float32)
    for core in range(N_CORES):
        b, g = divmod(core, G)
        for c in range(NCHUNK):
            out[b, CHUNK * c + 128 * g:CHUNK * c + 128 * (g + 1), :] = \
                results[core]["out"][128 * c:128 * (c + 1), :]
    return out


def kernel(**inputs):
    from concourse.bass_utils import run_bass_kernel_spmd

    in_maps = make_in_maps(inputs)
    nc = _get_nc()
    trace = bool(os.environ.get("KERNEL_TRACE"))
    res = run_bass_kernel_spmd(nc, in_maps, core_ids=list(range(N_CORES)),
                               trace=trace)
    if trace:
        _cache["exec_time_ns"] = res.exec_time_ns
    return assemble_out(res.results)


if __name__ == "__main__":
    nc = _get_nc()
    print("built OK; instructions:", len(nc.inst_map))
